# revision 3
# baseline (speedup 1.0000x reference)
"""Trainium2 Bass kernel for a channel-attention block.

Reference math (per batch sample, a: [C, N] with C=128 channels,
N = H*W spatial):
    b   = a @ a.T                  # [C, C] channel affinity (Gram)
    x   = softmax(b, axis=-1)
    c   = x @ a                    # [C, N]
    out = beta * c + a

Sharding: data-parallel over the batch dim - 16 samples / 8 cores =
2 samples per NeuronCore, no cross-core communication.

Kernel design (v2):
  * bf16 device I/O. The host casts `a` to bf16 before upload and
    upcasts the bf16 output; rel-err from rounding is ~2e-3, well
    inside the 2e-2 gate. This halves both HBM reads and writes:
    67 MB/core total vs 164 MB/core for the f32 baseline.
  * Full SBUF residency: each sample's 16 MiB of bf16 tiles stays in
    SBUF from stage A (Gram) through stage C (attend) - no second
    HBM read. The cache pool has n_loads + spare slots so sample s+1's
    loads can run ahead while stage C of sample s drains.
  * Residual fold: out = beta*softmax(b)@a + a == M @ a with
    M = diag(beta/rowsum) @ exp(b - rowmax) + I. Building M^T (128x128)
    per sample turns stage C into pure matmuls plus a PSUM->SBUF bf16
    copy; the per-element DVE epilogue of the baseline is gone.

Per-core pipeline (per sample):
  stage A: DMA bf16 [128, 2048] tiles into cache slots; PE-transpose
           each 128x128 block, DVE copies PSUM->SBUF, Gram matmuls
           accumulate b in one PSUM bank.
  fold:    rowmax (DVE), exp+rowsum (ACT), reciprocal, bs=beta/rowsum,
           G = bs*E (DVE), PE-transpose, M^T = G^T + I.
  stage C: out_tile = M^T.T @ cache_tile via bf16 matmuls; ACT/DVE
           copy-casts PSUM f32 -> bf16, store.
"""

import numpy as np

import concourse.bass as bass
import concourse.mybir as mybir
import concourse.tile as tile
from concourse import bacc
from concourse.bass_utils import run_bass_kernel_spmd
from concourse.masks import make_identity

F32 = mybir.dt.float32
BF16 = mybir.dt.bfloat16
NP_BF16 = mybir.dt.np(BF16)

N_CORES = 8
B, C, H, W = 16, 128, 256, 256
N_FULL = H * W
S = B // N_CORES  # samples per core


def build(S=S, C=C, N=N_FULL, load=2048, spare=8, mm_n=512,
          eng_atcopy="dve", eng_ccopy="alt", bufs=3):
    """Build + compile the per-core Bass program (bf16 in / bf16 out).

    Emission interleaves sample s's stage C with sample s+1's stage A;
    cache slots rotate so A(s+1) loads refill slots right after C(s)
    consumes them, with `spare` extra slots of prefetch headroom.
    """
    assert C == 128 and N % load == 0 and load % 512 == 0
    n_loads = N // load
    spare = min(spare, n_loads)
    TW = 512               # transpose-group width: 4 transposes per copy
    n_gram_mm = N // 128

    nc = bacc.Bacc("TRN2", target_bir_lowering=False, debug=False)

    a_d = nc.dram_tensor("a", [S, C, N], BF16, kind="ExternalInput").ap()
    beta_d = nc.dram_tensor("beta", [C, 1], F32, kind="ExternalInput").ap()
    out_d = nc.dram_tensor("out", [S, C, N], BF16, kind="ExternalOutput").ap()

    with tile.TileContext(nc) as tc:
        with (
            tc.tile_pool(name="const", bufs=1) as const_pool,
            tc.tile_pool(name="acache", bufs=n_loads + spare) as cache_pool,
            tc.tile_pool(name="at", bufs=4) as at_pool,
            tc.tile_pool(name="sm", bufs=2) as sm_pool,
            tc.tile_pool(name="cout", bufs=bufs) as cout_pool,
            tc.tile_pool(name="tp_ps", bufs=3, space="PSUM") as tp_psum,
            tc.tile_pool(name="gram_ps", bufs=2, space="PSUM") as gram_psum,
            tc.tile_pool(name="c_ps", bufs=3, space="PSUM") as c_psum,
        ):
            ident_bf = const_pool.tile([128, 128], BF16, tag="identbf")
            make_identity(nc, ident_bf)
            beta_sb = const_pool.tile([C, 1], F32, tag="beta")
            nc.sync.dma_start(beta_sb, beta_d)

            def copy_op(engine_sel, idx, out, in_):
                if engine_sel == "act" or (engine_sel == "alt" and idx % 2 == 0):
                    nc.scalar.copy(out, in_)
                else:
                    nc.vector.tensor_copy(out, in_)

            gram_state = {}   # s -> [b_ps, mm_count]
            mt_w = {}         # s -> M^T lhsT weights for stage C
            cached = {}       # (s, j) -> SBUF-resident bf16 a tile

            def stage_a_chunk(s, j):
                """Load bf16 tile j of sample s, transpose, Gram-accum."""
                if s not in gram_state:
                    b_ps = gram_psum.tile([C, C], F32, tag="gram",
                                          name=f"gram_{s}")
                    gram_state[s] = [b_ps, 0]
                st = gram_state[s]
                b_ps = st[0]
                a_t = cache_pool.tile([C, load], BF16, tag="acache",
                                      name=f"ac_{s}_{j}")
                cached[(s, j)] = a_t
                nc.sync.dma_start(a_t, a_d[s, :, j * load:(j + 1) * load])
                for g in range(load // TW):
                    src = a_t[:, g * TW:(g + 1) * TW]
                    tp = tp_psum.tile([128, TW], BF16, tag="tp",
                                      name=f"tp_{s}_{j}_{g}")
                    for q in range(TW // 128):
                        nc.tensor.transpose(
                            tp[:, q * 128:(q + 1) * 128],
                            src[:, q * 128:(q + 1) * 128],
                            ident_bf,
                        )
                    at_t = at_pool.tile([128, TW], BF16, tag="at",
                                        name=f"at_{s}_{j}_{g}")
                    copy_op(eng_atcopy, g, at_t, tp)
                    for q in range(TW // 128):
                        st[1] += 1
                        nc.tensor.matmul(
                            b_ps,
                            lhsT=at_t[:, q * 128:(q + 1) * 128],
                            rhs=at_t[:, q * 128:(q + 1) * 128],
                            start=(st[1] == 1),
                            stop=(st[1] == n_gram_mm),
                        )

            def build_m(s):
                """M^T = (diag(beta/rowsum) @ exp(b - rowmax))^T + I."""
                b_ps = gram_state[s][0]
                negm = sm_pool.tile([C, 1], F32, tag="negm", name=f"negm_{s}")
                nc.vector.tensor_reduce(
                    negm, b_ps, axis=mybir.AxisListType.X,
                    op=mybir.AluOpType.max, negate=True,
                )
                e_t = sm_pool.tile([C, C], F32, tag="e", name=f"e_{s}")
                ssum = sm_pool.tile([C, 1], F32, tag="ssum", name=f"ssum_{s}")
                nc.scalar.activation(
                    e_t, b_ps, mybir.ActivationFunctionType.Exp,
                    bias=negm, accum_out=ssum,
                )
                rec = sm_pool.tile([C, 1], F32, tag="rec", name=f"rec_{s}")
                nc.vector.reciprocal(rec, ssum)
                bs = sm_pool.tile([C, 1], F32, tag="bs", name=f"bs_{s}")
                nc.vector.tensor_scalar_mul(bs, rec, beta_sb)
                g_bf = sm_pool.tile([C, C], BF16, tag="g", name=f"g_{s}")
                nc.vector.tensor_scalar_mul(g_bf, e_t, bs)
                gt_ps = tp_psum.tile([128, TW], BF16, tag="tp", name=f"gt_{s}")
                nc.tensor.transpose(gt_ps[:, :128], g_bf, ident_bf)
                mt_sb = sm_pool.tile([C, C], BF16, tag="mt", name=f"mt_{s}")
                nc.vector.scalar_tensor_tensor(
                    out=mt_sb, in0=gt_ps[:, :128], scalar=1.0, in1=ident_bf,
                    op0=mybir.AluOpType.mult, op1=mybir.AluOpType.add,
                )
                mt_w[s] = mt_sb

            def stage_c_chunk(s, j):
                """out_tile = M^T.T @ a_tile (bf16), from the SBUF cache."""
                lhs_w = mt_w[s]
                a_t = cached.pop((s, j))
                c_out = cout_pool.tile([C, load], BF16, tag="cout",
                                       name=f"cout_{s}_{j}")
                for q in range(load // mm_n):
                    sl = slice(q * mm_n, (q + 1) * mm_n)
                    c_ps = c_psum.tile([128, mm_n], F32, tag="cps",
                                       name=f"cps_{s}_{j}_{q}")
                    nc.tensor.matmul(
                        c_ps, lhsT=lhs_w, rhs=a_t[:, sl], start=True, stop=True,
                    )
                    copy_op(eng_ccopy, q + j, c_out[:, sl], c_ps)
                nc.scalar.dma_start(out_d[s, :, j * load:(j + 1) * load], c_out)

            # Software-pipelined emission across samples:
            #   A(0); M(0); [C(0,j) x A(1,j)]; M(1); C(1)
            for j in range(n_loads):
                stage_a_chunk(0, j)
            build_m(0)
            for s in range(1, S):
                for j in range(n_loads):
                    stage_c_chunk(s - 1, j)
                    stage_a_chunk(s, j)
                build_m(s)
            for j in range(n_loads):
                stage_c_chunk(S - 1, j)

    nc.compile()
    return nc


_NC_CACHE: dict = {}


def _get_nc(**kw):
    key = tuple(sorted(kw.items()))
    if key not in _NC_CACHE:
        _NC_CACHE[key] = build(**kw)
    return _NC_CACHE[key]


def kernel(a, beta):
    """Full-input entry point: a [16,128,256,256] f32, beta [1] f32."""
    a = np.asarray(a)
    beta = np.asarray(beta, dtype=np.float32)
    nb, ch, h, w = a.shape
    n = h * w
    s = nb // N_CORES
    a_bf = np.ascontiguousarray(a.reshape(nb, ch, n)).astype(NP_BF16)
    beta_b = np.broadcast_to(beta.reshape(1, 1), (ch, 1)).copy()

    nc = _get_nc(S=s, C=ch, N=n)
    in_maps = [
        {"a": a_bf[i * s:(i + 1) * s], "beta": beta_b} for i in range(N_CORES)
    ]
    res = run_bass_kernel_spmd(nc, in_maps, list(range(N_CORES)))
    out = np.concatenate([res.results[i]["out"] for i in range(N_CORES)], axis=0)
    return out.astype(np.float32).reshape(nb, ch, h, w)


# revision 5
# speedup vs baseline: 1.0624x; 1.0624x over previous
"""Trainium2 Bass kernel for a channel-attention block.

Reference math (per batch sample, a: [C, N] with C=128 channels,
N = H*W spatial):
    b   = a @ a.T                  # [C, C] channel affinity (Gram)
    x   = softmax(b, axis=-1)
    c   = x @ a                    # [C, N]
    out = beta * c + a

Sharding: data-parallel over the batch dim - 16 samples / 8 cores =
2 samples per NeuronCore, no cross-core communication.

Kernel design (v2):
  * bf16 device I/O. The host casts `a` to bf16 before upload and
    upcasts the bf16 output; rel-err from rounding is ~3e-3, well
    inside the 2e-2 gate. This halves both HBM reads and writes:
    67 MB/core total vs 164 MB/core for the f32 baseline.
  * Full SBUF residency: each sample's 16 MiB of bf16 tiles stays in
    SBUF from stage A (Gram) through stage C (attend) - no second
    HBM read. The cache pool has n_loads + spare slots so sample s+1's
    loads can run ahead while stage C of sample s drains.
  * Residual fold: out = beta*softmax(b)@a + a == M @ a with
    M = diag(beta/rowsum) @ exp(b - rowmax) + I. Building M^T (128x128)
    per sample turns stage C into pure matmuls plus a PSUM->SBUF bf16
    cast; the per-element DVE epilogue of the baseline is gone.
  * Phase-aware engine routing: the kernel alternates PSUM->SBUF
    copies between DVE and ACT per-phase so neither becomes the gate;
    stage C of the last sample stores via the (then idle) sync queue.

Per-core pipeline (per sample):
  stage A: DMA bf16 [128, 2048] tiles into cache slots; PE-transpose
           each 128x128 block, DVE/ACT copy PSUM->SBUF, Gram matmuls
           accumulate b in one PSUM bank.
  fold:    rowmax (DVE), exp+rowsum (ACT), reciprocal, bs=beta/rowsum,
           G = bs*E (DVE), PE-transpose, M^T = G^T + I.
  stage C: out_tile = M^T.T @ cache_tile via 1024-wide bf16 matmuls;
           ACT/DVE copy-cast PSUM f32 -> bf16, store.
"""

import numpy as np

import concourse.bass as bass
import concourse.mybir as mybir
import concourse.tile as tile
from concourse import bacc
from concourse.bass_utils import run_bass_kernel_spmd
from concourse.masks import make_identity

F32 = mybir.dt.float32
BF16 = mybir.dt.bfloat16
NP_BF16 = mybir.dt.np(BF16)

N_CORES = 8
B, C, H, W = 16, 128, 256, 256
N_FULL = H * W
S = B // N_CORES  # samples per core


def build(S=S, C=C, N=N_FULL, load=2048, spare=8, mm_n=1024, tw=1024,
          bufs=3):
    """Build + compile the per-core Bass program (bf16 in / bf16 out)."""
    assert C == 128 and N % load == 0 and load % tw == 0 and load % mm_n == 0
    n_loads = N // load
    spare = min(spare, n_loads)
    n_gram_mm = N // 128

    nc = bacc.Bacc("TRN2", target_bir_lowering=False, debug=False)

    a_d = nc.dram_tensor("a", [S, C, N], BF16, kind="ExternalInput").ap()
    beta_d = nc.dram_tensor("beta", [C, 1], F32, kind="ExternalInput").ap()
    out_d = nc.dram_tensor("out", [S, C, N], BF16, kind="ExternalOutput").ap()

    with tile.TileContext(nc) as tc:
        with (
            tc.tile_pool(name="const", bufs=1) as const_pool,
            tc.tile_pool(name="acache", bufs=n_loads + spare) as cache_pool,
            tc.tile_pool(name="at", bufs=4) as at_pool,
            tc.tile_pool(name="sm", bufs=2) as sm_pool,
            tc.tile_pool(name="cout", bufs=bufs) as cout_pool,
            tc.tile_pool(name="tp_ps", bufs=3, space="PSUM") as tp_psum,
            tc.tile_pool(name="gram_ps", bufs=1, space="PSUM") as gram_psum,
            tc.tile_pool(name="c_ps", bufs=2, space="PSUM") as c_psum,
        ):
            ident_bf = const_pool.tile([128, 128], BF16, tag="identbf")
            make_identity(nc, ident_bf)
            beta_sb = const_pool.tile([C, 1], F32, tag="beta")
            nc.sync.dma_start(beta_sb, beta_d)

            def copy_op(eng, out, in_):
                if eng == "act":
                    nc.scalar.copy(out, in_)
                elif eng == "gps":
                    nc.gpsimd.tensor_copy(out, in_)
                else:
                    nc.vector.tensor_copy(out, in_)

            gram_state = {}   # s -> [b_ps, mm_count]
            mt_w = {}         # s -> M^T lhsT weights for stage C
            cached = {}       # (s, j) -> SBUF-resident bf16 a tile

            def stage_a_chunk(s, j, at_engs=("dve", "act")):
                """Load bf16 tile j of sample s, transpose, Gram-accum."""
                if s not in gram_state:
                    b_ps = gram_psum.tile([C, C], F32, tag="gram",
                                          name=f"gram_{s}")
                    gram_state[s] = [b_ps, 0]
                st = gram_state[s]
                b_ps = st[0]
                a_t = cache_pool.tile([C, load], BF16, tag="acache",
                                      name=f"ac_{s}_{j}")
                cached[(s, j)] = a_t
                nc.sync.dma_start(a_t, a_d[s, :, j * load:(j + 1) * load])
                for g in range(load // tw):
                    src = a_t[:, g * tw:(g + 1) * tw]
                    tp = tp_psum.tile([128, tw], BF16, tag="tp",
                                      name=f"tp_{s}_{j}_{g}")
                    for q in range(tw // 128):
                        nc.tensor.transpose(
                            tp[:, q * 128:(q + 1) * 128],
                            src[:, q * 128:(q + 1) * 128],
                            ident_bf,
                        )
                    at_t = at_pool.tile([128, tw], BF16, tag="at",
                                        name=f"at_{s}_{j}_{g}")
                    copy_op(at_engs[g % len(at_engs)], at_t, tp)
                    for q in range(tw // 128):
                        st[1] += 1
                        nc.tensor.matmul(
                            b_ps,
                            lhsT=at_t[:, q * 128:(q + 1) * 128],
                            rhs=at_t[:, q * 128:(q + 1) * 128],
                            start=(st[1] == 1),
                            stop=(st[1] == n_gram_mm),
                        )

            def build_m(s):
                """M^T = (diag(beta/rowsum) @ exp(b - rowmax))^T + I."""
                b_ps = gram_state[s][0]
                negm = sm_pool.tile([C, 1], F32, tag="negm", name=f"negm_{s}")
                nc.vector.tensor_reduce(
                    negm, b_ps, axis=mybir.AxisListType.X,
                    op=mybir.AluOpType.max, negate=True,
                )
                e_t = sm_pool.tile([C, C], F32, tag="e", name=f"e_{s}")
                ssum = sm_pool.tile([C, 1], F32, tag="ssum", name=f"ssum_{s}")
                nc.scalar.activation(
                    e_t, b_ps, mybir.ActivationFunctionType.Exp,
                    bias=negm, accum_out=ssum,
                )
                rec = sm_pool.tile([C, 1], F32, tag="rec", name=f"rec_{s}")
                nc.vector.reciprocal(rec, ssum)
                bs = sm_pool.tile([C, 1], F32, tag="bs", name=f"bs_{s}")
                nc.vector.tensor_scalar_mul(bs, rec, beta_sb)
                g_bf = sm_pool.tile([C, C], BF16, tag="g", name=f"g_{s}")
                nc.vector.tensor_scalar_mul(g_bf, e_t, bs)
                gt_ps = tp_psum.tile([128, tw], BF16, tag="tp", name=f"gt_{s}")
                nc.tensor.transpose(gt_ps[:, :128], g_bf, ident_bf)
                mt_sb = sm_pool.tile([C, C], BF16, tag="mt", name=f"mt_{s}")
                nc.vector.scalar_tensor_tensor(
                    out=mt_sb, in0=gt_ps[:, :128], scalar=1.0, in1=ident_bf,
                    op0=mybir.AluOpType.mult, op1=mybir.AluOpType.add,
                )
                mt_w[s] = mt_sb

            def stage_c_chunk(s, j, c_engs=("dve", "act"), st_q=nc.scalar):
                """out_tile = M^T.T @ a_tile (bf16), from the SBUF cache.

                Each [128, mm_n] PSUM tile takes mm_n//512 matmuls (one per
                512-f32 PSUM bank; a single matmul cannot cross banks) and
                drains with one wide cast-copy to SBUF bf16.
                """
                lhs_w = mt_w[s]
                a_t = cached.pop((s, j))
                c_out = cout_pool.tile([C, load], BF16, tag="cout",
                                       name=f"cout_{s}_{j}")
                for g in range(load // mm_n):
                    gsl = slice(g * mm_n, (g + 1) * mm_n)
                    c_ps = c_psum.tile([128, mm_n], F32, tag="cps",
                                       name=f"cps_{s}_{j}_{g}")
                    for q in range(mm_n // 512):
                        sl = slice(g * mm_n + q * 512, g * mm_n + (q + 1) * 512)
                        nc.tensor.matmul(
                            c_ps[:, q * 512:(q + 1) * 512], lhsT=lhs_w,
                            rhs=a_t[:, sl], start=True, stop=True,
                        )
                    copy_op(c_engs[g % len(c_engs)], c_out[:, gsl], c_ps)
                st_q.dma_start(out_d[s, :, j * load:(j + 1) * load], c_out)

            # Software-pipelined emission across samples:
            #   A(0); M(0); [C(0,j) x A(1,j)]; M(1); C(1)
            for j in range(n_loads):
                stage_a_chunk(0, j, at_engs=("dve", "act"))
            build_m(0)
            for s in range(1, S):
                for j in range(n_loads):
                    stage_c_chunk(s - 1, j, c_engs=("act", "dve"),
                                  st_q=nc.scalar)
                    stage_a_chunk(s, j, at_engs=("dve", "dve"))
                build_m(s)
            for j in range(n_loads):
                stage_c_chunk(S - 1, j, c_engs=("dve", "act"), st_q=nc.sync)

    nc.compile()
    return nc


_NC_CACHE: dict = {}


def _get_nc(**kw):
    key = tuple(sorted(kw.items()))
    if key not in _NC_CACHE:
        _NC_CACHE[key] = build(**kw)
    return _NC_CACHE[key]


def kernel(a, beta):
    """Full-input entry point: a [16,128,256,256] f32, beta [1] f32."""
    a = np.asarray(a)
    beta = np.asarray(beta, dtype=np.float32)
    nb, ch, h, w = a.shape
    n = h * w
    s = nb // N_CORES
    a_bf = np.ascontiguousarray(a.reshape(nb, ch, n)).astype(NP_BF16)
    beta_b = np.broadcast_to(beta.reshape(1, 1), (ch, 1)).copy()

    nc = _get_nc(S=s, C=ch, N=n)
    in_maps = [
        {"a": a_bf[i * s:(i + 1) * s], "beta": beta_b} for i in range(N_CORES)
    ]
    res = run_bass_kernel_spmd(nc, in_maps, list(range(N_CORES)))
    out = np.concatenate([res.results[i]["out"] for i in range(N_CORES)], axis=0)
    return out.astype(np.float32).reshape(nb, ch, h, w)
